# revision 1
# baseline (speedup 1.0000x reference)
"""GCN layer (x @ K scattered over edges) on 8 Trainium2 NeuronCores.

out[n, :] = (sum_{e: dst[e]==n} attr[e] * x[src[e], :]) @ K + bias

Sharding: destination nodes are split across the 8 cores (12500 each, padded
to 98 tiles of 128). Each core indirect-DMA-gathers x[src] rows for its own
edges straight from its DRAM copy of x (replicated), scatter-accumulates them
per 128-node tile with a one-hot matmul in PSUM, and applies the 128x128
projection + bias after aggregation (projection commutes with segment-sum).
No collectives needed. The same NEFF runs on all cores; the per-tile chunk
schedule is the max chunk count over cores so shapes agree.

Self-contained: hardcodes the problem shape (N=100000, D=128, E=600000).
"""
import sys

sys.path.insert(0, '/opt/trn_rl_repo')

import numpy as np
import concourse.bacc as bacc
import concourse.bass as bass
import concourse.mybir as mybir
import concourse.tile as tile
from concourse.bass_utils import run_bass_kernel_spmd

F32 = mybir.dt.float32
I32 = mybir.dt.int32
P = 128
D = 128
N_CORES = 8
N = 100000
NODES_PER_CORE = N // N_CORES            # 12500
N_TILES = (NODES_PER_CORE + P - 1) // P  # 98 (last tile: 84 real nodes)


def _build_nc(V, schedule, gx_bufs=16, psum_bufs=4, osb_bufs=3, aggsb_bufs=3):
    n_tiles = len(schedule)
    C = int(sum(schedule))

    nc = bacc.Bacc(trn_type="TRN2", target_bir_lowering=False, debug=False,
                   enable_asserts=True, num_devices=N_CORES)
    x = nc.dram_tensor("x", [V, D], F32, kind="ExternalInput")
    meta = nc.dram_tensor("meta", [P, 3 * C], I32, kind="ExternalInput")
    kb = nc.dram_tensor("kb", [P, D + 1], F32, kind="ExternalInput")
    out_t = nc.dram_tensor("out_t", [P, n_tiles * P], F32, kind="ExternalOutput")

    with tile.TileContext(nc) as tc:
        with tc.tile_pool(name="const", bufs=1) as cpool, \
             tc.tile_pool(name="gx", bufs=gx_bufs) as gxp, \
             tc.tile_pool(name="s", bufs=gx_bufs) as sp, \
             tc.tile_pool(name="aggsb", bufs=aggsb_bufs) as aggsbp, \
             tc.tile_pool(name="osb", bufs=osb_bufs) as osbp, \
             tc.tile_pool(name="psum", bufs=psum_bufs, space="PSUM") as pp, \
             tc.tile_pool(name="psum2", bufs=2, space="PSUM") as pp2:

            iota_i = cpool.tile([P, P], I32)
            nc.gpsimd.iota(iota_i[:], [[1, P]], channel_multiplier=0)
            iota_f = cpool.tile([P, P], F32)
            nc.vector.tensor_copy(iota_f[:], iota_i[:])

            kb_sb = cpool.tile([P, D + 1], F32)
            nc.sync.dma_start(out=kb_sb[:], in_=kb[:])
            meta_t = cpool.tile([P, 3 * C], I32)
            nc.sync.dma_start(out=meta_t[:], in_=meta[:])

            ci = 0
            for t in range(n_tiles):
                nch = schedule[t]
                aggT = pp.tile([P, P], F32)
                for j in range(nch):
                    gx = gxp.tile([P, D], F32)
                    nc.gpsimd.indirect_dma_start(
                        out=gx[:], out_offset=None, in_=x[:],
                        in_offset=bass.IndirectOffsetOnAxis(
                            ap=meta_t[:, ci:ci + 1], axis=0),
                    )
                    s_tile = sp.tile([P, P], F32)
                    # S[e, n] = attr[e] * (dstloc[e] == n)
                    nc.vector.tensor_scalar(
                        out=s_tile[:], in0=iota_f[:],
                        scalar1=meta_t[:, C + ci:C + ci + 1].bitcast(F32),
                        scalar2=meta_t[:, 2 * C + ci:2 * C + ci + 1].bitcast(F32),
                        op0=mybir.AluOpType.is_equal,
                        op1=mybir.AluOpType.mult,
                    )
                    # aggT[k, n] += sum_e gx[e, k] * S[e, n]
                    nc.tensor.matmul(
                        out=aggT[:], lhsT=gx[:], rhs=s_tile[:],
                        start=(j == 0), stop=(j == nch - 1),
                    )
                    ci += 1

                aggT_sb = aggsbp.tile([P, P], F32)
                nc.scalar.activation(
                    out=aggT_sb[:], in_=aggT[:],
                    func=mybir.ActivationFunctionType.Identity)
                o_psum = pp2.tile([P, P], F32)
                # o[d, n] = sum_k K[k, d] * aggT[k, n]
                nc.tensor.matmul(out=o_psum[:], lhsT=kb_sb[:, :D],
                                 rhs=aggT_sb[:], start=True, stop=True)
                o_sb = osbp.tile([P, P], F32)
                # + bias[d] (per-partition; the tile is [d, n])
                nc.vector.tensor_scalar(
                    out=o_sb[:], in0=o_psum[:],
                    scalar1=kb_sb[:, D:D + 1], scalar2=None,
                    op0=mybir.AluOpType.add,
                )
                nc.sync.dma_start(out=out_t[:, t * P:(t + 1) * P], in_=o_sb[:])
            assert ci == C
    nc.finalize()
    return nc


def _prepare(x, edge_indices, edge_attr):
    """Partition + sort edges by (core, tile); build the shared chunk
    schedule and each core's packed metadata array."""
    src = np.ascontiguousarray(edge_indices[0]).astype(np.int64)
    dst = np.ascontiguousarray(edge_indices[1]).astype(np.int64)
    attr = np.ascontiguousarray(edge_attr).astype(np.float32)

    core = dst // NODES_PER_CORE
    local = dst - core * NODES_PER_CORE
    tl = local // P
    dloc = local - tl * P

    key = core * N_TILES + tl
    order = np.argsort(key, kind="stable")
    key_s = key[order]
    src_s = src[order].astype(np.int32)
    dloc_s = dloc[order].astype(np.float32)
    attr_s = attr[order]

    counts = np.bincount(key_s, minlength=N_CORES * N_TILES) \
        .reshape(N_CORES, N_TILES)
    schedule = np.maximum(1, (counts.max(axis=0) + P - 1) // P).astype(np.int64)
    C = int(schedule.sum())
    chunk_base = np.concatenate([[0], np.cumsum(schedule)[:-1]])

    group_start = np.concatenate([[0], np.cumsum(counts.reshape(-1))[:-1]])
    within = np.arange(len(key_s)) - group_start[key_s]
    col = chunk_base[key_s % N_TILES] + within // P
    row = within % P

    metas = []
    e_core = core[order]
    for c in range(N_CORES):
        m = e_core == c
        src_arr = np.zeros((P, C), np.int32)
        dst_arr = np.zeros((P, C), np.float32)
        attr_arr = np.zeros((P, C), np.float32)
        src_arr[row[m], col[m]] = src_s[m]
        dst_arr[row[m], col[m]] = dloc_s[m]
        attr_arr[row[m], col[m]] = attr_s[m]
        metas.append(np.concatenate(
            [src_arr, dst_arr.view(np.int32), attr_arr.view(np.int32)], axis=1))
    return schedule, metas


def _run(x, edge_indices, edge_attr, kernel, bias, trace=False):
    xv = np.ascontiguousarray(x, dtype=np.float32)
    kbm = np.concatenate(
        [np.ascontiguousarray(kernel, dtype=np.float32),
         np.ascontiguousarray(bias, dtype=np.float32).reshape(D, 1)], axis=1)
    schedule, metas = _prepare(xv, edge_indices, edge_attr)
    nc = _build_nc(N, [int(s) for s in schedule])
    in_maps = [{"x": xv, "meta": metas[c], "kb": kbm} for c in range(N_CORES)]
    res = run_bass_kernel_spmd(nc, in_maps, core_ids=list(range(N_CORES)),
                               trace=trace)
    parts = [res.results[c]["out_t"].T[:NODES_PER_CORE] for c in range(N_CORES)]
    out = np.concatenate(parts, axis=0)
    return out, res


def kernel(x, edge_indices, edge_attr, kernel, bias):
    out, _ = _run(x, edge_indices, edge_attr, kernel, bias, trace=False)
    return out


def kernel_traced(x, edge_indices, edge_attr, kernel, bias):
    out, res = _run(x, edge_indices, edge_attr, kernel, bias, trace=True)
    return out, res
